# revision 5
# baseline (speedup 1.0000x reference)
"""GNN message-passing kernel v2 for Trainium2, 8 NeuronCores.

v1 gathered per-node snp rows with dma_gather: 125k descriptors/core at
~8.5ns/descriptor (descriptor-bound) = ~1.06ms. v2 eliminates per-node
descriptors entirely by flipping the contraction order:

  h1[d,b] = sum_n W1[d, n2g[n]] * v[b, ids[n]]     (v = snp * fbar)
          = sum_s u[d,s] * v[b,s],   u[d,s] = sum_{n: ids[n]=s} W1[d, n2g[n]]

Nodes are grouped by snp id s (host integer sort). u's summands are
columns of the *parameter* W1, so the host pre-expands them into a
padded run table E (pure relayout of weights by a host-computed index
permutation, same category as v1's W1 row permutation). The device:

  1. builds v[s,b] = snpT[s,:] * fbar[s] in SBUF (affine DMA in)
  2. streams E (bf16, [128, 64*l_total]) with ~10 big affine DMAs
  3. DVE segment-reduce contiguous runs -> u blocks [128, 64]
  4. PE: h1[64,B] += u_blk[128,64].T @ v_blk[128,B]   (196 blocks)
  5. AllReduce h1 over 8 cores; BN+relu; W2; BN+relu; [:15]; W3 -> [1,B]

Sharding: s-range per core (25000 snp rows); each node incidence is
processed on exactly one core; only [64,B] is all-reduced.

Host does ONLY integer/layout work (sorting, padding, permutation,
weight relayout/dtype cast, zero fill).
"""

import numpy as np
import ml_dtypes

import concourse.bacc as bacc
import concourse.bass as bass
import concourse.tile as tile
from concourse import mybir
from concourse.bass_utils import run_bass_kernel_spmd

F32 = mybir.dt.float32
BF16 = mybir.dt.bfloat16

B = 16
S = 200000
G = 20000
N = 1000000
F = 8
D = 64
FEAT = 16
MAIN = 15
BN_EPS = 1e-5
CORES = 8

SPC = S // CORES          # snp rows per core (25000)
PPART = 128
SBLK = (SPC + PPART - 1) // PPART   # 196 s-blocks per core
TROWS = PPART * SBLK      # 25088 padded s-slots per core
NCHUNK = 10               # E streamed in this many DMAs


def _f32_to_bf16_bits(x):
    """Round-to-nearest-even f32 -> bf16, kept as uint16 bit pattern."""
    u = np.asarray(x, np.float32).view(np.uint32)
    rounded = (u + 0x7FFF + ((u >> 16) & 1)) >> 16
    return rounded.astype(np.uint16)


# --------------------------------------------------------------------------
# host-side packing (integers / layout only)
# --------------------------------------------------------------------------

def prepare(snp, filters, W1, b1, g1, bb1, W2, b2, g2, bb2, W3, b3,
            snp_ids, node2gene):
    snp = np.asarray(snp, dtype=np.float32)
    filters = np.asarray(filters, dtype=np.float32)
    ids = np.asarray(snp_ids, dtype=np.int64)
    n2g = np.asarray(node2gene, dtype=np.int64)

    snpT = np.ascontiguousarray(snp.T)          # [S, B]
    filtT = np.ascontiguousarray(filters.T)     # [S, F]
    w1_bits = _f32_to_bf16_bits(np.asarray(W1, np.float32))  # [D, G] uint16

    core_of = ids // SPC
    s_local = (ids % SPC).astype(np.int64)

    # group node incidences by (core, s_local); gene of each node rides along
    order = np.argsort(core_of * np.int64(SPC) + s_local, kind="stable")
    oc = core_of[order]
    osl = s_local[order]
    og = n2g[order]
    core_bounds = np.searchsorted(oc, np.arange(CORES + 1))

    # per-core per-slot counts and count-sorted slot permutation
    counts = np.zeros((CORES, SPC), np.int64)
    perms = []
    sorted_counts = []
    for c in range(CORES):
        lo, hi = core_bounds[c], core_bounds[c + 1]
        cnt = np.bincount(osl[lo:hi], minlength=SPC)
        counts[c] = cnt
        pi = np.argsort(-cnt, kind="stable")    # s-slots by count desc
        perms.append(pi)
        sorted_counts.append(cnt[pi])

    # shared block run-lengths L_t = max over cores of block head count
    block_l = []
    for t in range(SBLK):
        m = max(int(sorted_counts[c][t * PPART]) for c in range(CORES))
        block_l.append(max(m, 1))
    l_total = int(sum(block_l))

    # chunk boundaries: consecutive blocks, ~even free-dim bytes per chunk
    offs = np.concatenate([[0], np.cumsum(block_l)])  # block start cols
    target = l_total / NCHUNK
    chunks = []  # list of (blk_lo, blk_hi)
    lo = 0
    for k in range(1, NCHUNK):
        hi = int(np.searchsorted(offs, target * k))
        hi = min(max(hi, lo + 1), SBLK - (NCHUNK - k))
        chunks.append((lo, hi))
        lo = hi
    chunks.append((lo, SBLK))

    in_maps = []
    for c in range(CORES):
        lo, hi = core_bounds[c], core_bounds[c + 1]
        gsl = osl[lo:hi]
        gg = og[lo:hi]
        slot_start = np.searchsorted(gsl, np.arange(SPC + 1))
        pi = perms[c]
        cnt = counts[c]

        # E: [128, 64 * l_total] uint16(bf16 bits); slot (t,p) <- pi[t*128+p]
        # vectorized fill: per block, gene-index matrix [128, lt] with pad
        # slots pointing at an appended all-zero W1 column G
        w1_ext = np.concatenate(
            [w1_bits, np.zeros((D, 1), np.uint16)], axis=1)  # [D, G+1]
        pi_pad = np.concatenate([pi, np.zeros(TROWS - SPC, np.int64)])
        slot_s = pi_pad.reshape(SBLK, PPART)
        # counts in permuted slot order; pad slots are zero
        slot_n = np.concatenate(
            [cnt[pi], np.zeros(TROWS - SPC, np.int64)]).reshape(SBLK, PPART)
        E = np.zeros((PPART, D * l_total), np.uint16)
        for t in range(SBLK):
            lt = block_l[t]
            off = int(offs[t]) * D
            starts = slot_start[slot_s[t]]              # [128]
            lens = slot_n[t]                            # [128]
            col = np.arange(lt)
            mask = col[None, :] < lens[:, None]
            idxm = starts[:, None] + np.minimum(col[None, :],
                                                np.maximum(lens[:, None] - 1, 0))
            idxm = np.clip(idxm, 0, max(len(gg) - 1, 0))
            gidx = np.where(mask, gg[idxm] if len(gg) else G, G)  # [128, lt]
            blk = w1_ext[:, gidx]                       # [D, 128, lt]
            E[:, off:off + D * lt] = (
                blk.transpose(1, 0, 2).reshape(PPART, D * lt))

        # v source: snpT/filtT rows of this core in pi order, block-major
        # device reads rearrange("(t p) b -> p t b")
        sl = slice(c * SPC, (c + 1) * SPC)
        snp_loc = snpT[sl]                      # [SPC, B]
        filt_loc = filtT[sl]                    # [SPC, F]
        snp_pad = np.zeros((TROWS, B), np.float32)
        snp_pad[:SPC] = snp_loc[pi]
        filt_pad = np.zeros((TROWS, F), np.float32)
        filt_pad[:SPC] = filt_loc[pi]

        in_maps.append(dict(
            snpT=snp_pad,
            filtT=filt_pad,
            E=E.view(ml_dtypes.bfloat16),
            b1=np.asarray(b1, np.float32).reshape(D, 1),
            g1=np.asarray(g1, np.float32).reshape(D, 1),
            bb1=np.asarray(bb1, np.float32).reshape(D, 1),
            w2t=np.ascontiguousarray(np.asarray(W2, np.float32).T),
            b2=np.asarray(b2, np.float32).reshape(FEAT, 1),
            g2=np.asarray(g2, np.float32).reshape(FEAT, 1),
            bb2=np.asarray(bb2, np.float32).reshape(FEAT, 1),
            w3t=np.ascontiguousarray(np.asarray(W3, np.float32).T),
            b3=np.asarray(b3, np.float32).reshape(1, 1),
        ))
    meta = dict(block_l=block_l, l_total=l_total, chunks=chunks)
    return in_maps, meta


# --------------------------------------------------------------------------
# device program
# --------------------------------------------------------------------------

def build_program(meta, reps=1):
    block_l = meta["block_l"]
    l_total = meta["l_total"]
    chunks = meta["chunks"]
    offs = [0]
    for lt in block_l:
        offs.append(offs[-1] + lt)

    nc = bacc.Bacc("TRN2", target_bir_lowering=False, debug=False,
                   num_devices=CORES)

    def din(name, shape, dt=F32):
        return nc.dram_tensor(name, shape, dt, kind="ExternalInput").ap()

    snpT = din("snpT", [TROWS, B])
    filtT = din("filtT", [TROWS, F])
    E = din("E", [PPART, D * l_total], BF16)
    b1 = din("b1", [D, 1]); g1 = din("g1", [D, 1]); bb1 = din("bb1", [D, 1])
    w2t = din("w2t", [D, FEAT])
    b2 = din("b2", [FEAT, 1]); g2 = din("g2", [FEAT, 1]); bb2 = din("bb2", [FEAT, 1])
    w3t = din("w3t", [MAIN, 1]); b3 = din("b3", [1, 1])
    out = nc.dram_tensor("out", [1, B], F32, kind="ExternalOutput").ap()

    with tile.TileContext(nc) as tc:
        with (
            tc.tile_pool(name="eচunk", bufs=3) as ep,
            tc.tile_pool(name="u", bufs=3) as up,
            tc.tile_pool(name="small", bufs=4) as sp,
            tc.tile_pool(name="singles", bufs=1) as singles,
            tc.tile_pool(name="psum", bufs=2, space="PSUM") as pp,
            tc.tile_pool(name="dram", bufs=1, space="DRAM") as dp,
        ):
            for rep in range(reps):
                # ---- 1. build v = snpT * fbar in SBUF [128, SBLK, B] ----
                snp_sb = singles.tile([PPART, SBLK, B], F32, tag="snp")
                fil_sb = singles.tile([PPART, SBLK, F], F32, tag="fil")
                fbar = singles.tile([PPART, SBLK], F32, tag="fbar")
                nc.sync.dma_start(
                    out=snp_sb[:], in_=snpT.rearrange("(t p) b -> p t b", p=PPART))
                nc.sync.dma_start(
                    out=fil_sb[:], in_=filtT.rearrange("(t p) f -> p t f", p=PPART))
                nc.vector.tensor_reduce(
                    out=fbar[:], in_=fil_sb[:],
                    axis=mybir.AxisListType.X, op=mybir.AluOpType.add)
                v_sb = singles.tile([PPART, SBLK, B], F32, tag="v")
                nc.vector.tensor_scalar(
                    out=fbar[:], in0=fbar[:], scalar1=1.0 / F, scalar2=None,
                    op0=mybir.AluOpType.mult)
                nc.vector.tensor_tensor(
                    out=v_sb[:], in0=snp_sb[:],
                    in1=fbar[:].unsqueeze(-1).to_broadcast([PPART, SBLK, B]),
                    op=mybir.AluOpType.mult)

                # ---- 2+3+4. stream E, segment-reduce, matmul ----
                h1_ps = pp.tile([D, B], F32, tag="h1")
                for ci, (blo, bhi) in enumerate(chunks):
                    c0 = offs[blo] * D
                    c1 = offs[bhi] * D
                    et = ep.tile([PPART, c1 - c0], BF16, tag="et")
                    nc.sync.dma_start(out=et[:], in_=E[:, c0:c1])
                    # fuse consecutive blocks with equal L into one reduce
                    bi = blo
                    while bi < bhi:
                        lt = block_l[bi]
                        nb = 1
                        while bi + nb < bhi and block_l[bi + nb] == lt:
                            nb += 1
                        u_t = up.tile([PPART, nb, D], F32, tag="u")
                        src = et[:, offs[bi] * D - c0:offs[bi + nb] * D - c0]
                        src = src.rearrange("p (n e l) -> p n e l", n=nb, e=D)
                        nc.vector.tensor_reduce(
                            out=u_t[:], in_=src,
                            axis=mybir.AxisListType.X, op=mybir.AluOpType.add)
                        for i in range(nb):
                            t = bi + i
                            nc.tensor.matmul(
                                out=h1_ps[:],
                                lhsT=u_t[:, i, :],
                                rhs=v_sb[:, t, :],
                                start=(t == 0), stop=(t == SBLK - 1))
                        bi += nb

                # ---- 5. all-reduce + MLP tail ----
                # b1/b2 are dropped: training-mode BN subtracts the batch
                # mean, so a constant per-feature shift before BN cancels.
                h1_sb = singles.tile([D, B], F32, tag="h1sb")
                nc.vector.tensor_copy(out=h1_sb[:], in_=h1_ps[:])
                cc_in = dp.tile([D, B], F32)
                cc_out = dp.tile([D, B], F32)
                nc.gpsimd.dma_start(out=cc_in[:], in_=h1_sb[:])
                nc.gpsimd.collective_compute(
                    "AllReduce", mybir.AluOpType.add,
                    replica_groups=[list(range(CORES))],
                    ins=[cc_in.opt()], outs=[cc_out.opt()])
                h1 = singles.tile([D, B], F32, tag="h1f")
                nc.sync.dma_start(out=h1[:], in_=cc_out[:])

                small = [(g1, D), (bb1, D), (g2, FEAT), (bb2, FEAT), (b3, 1)]
                sb = {}
                for ap_, p in small:
                    t_ = singles.tile([p, 1], F32, tag=f"sm_{ap_.tensor.name}")
                    nc.sync.dma_start(out=t_[:], in_=ap_[:])
                    sb[ap_.tensor.name] = t_
                w2t_sb = singles.tile([D, FEAT], F32, tag="w2t")
                nc.sync.dma_start(out=w2t_sb[:], in_=w2t[:])
                w3t_sb = singles.tile([MAIN, 1], F32, tag="w3t")
                nc.sync.dma_start(out=w3t_sb[:], in_=w3t[:])
                eps1 = singles.tile([PPART, 1], F32, tag="eps")
                nc.vector.memset(eps1[:], BN_EPS)

                def batchnorm_relu(x_in, x_out, gamma, beta, p):
                    # x_out = relu(gamma * (x_in - mu) * rsqrt(var+eps) + beta)
                    stats = sp.tile([PPART, 6], F32, tag="bnstats")
                    mv = sp.tile([PPART, 2], F32, tag="bnmv")
                    nc.vector.bn_stats(out=stats[:p, :], in_=x_in[:])
                    nc.vector.bn_aggr(out=mv[:p, :], in_=stats[:p, :])
                    inv = sp.tile([PPART, 1], F32, tag="bninv")
                    nc.scalar.activation(
                        out=inv[:p, :], in_=mv[:p, 1:2],
                        func=mybir.ActivationFunctionType.Sqrt,
                        bias=eps1[:p, :], scale=1.0)
                    nc.vector.reciprocal(out=inv[:p, :], in_=inv[:p, :])
                    nc.vector.tensor_scalar(
                        out=x_out[:], in0=x_in[:], scalar1=mv[:p, 0:1],
                        scalar2=inv[:p, :],
                        op0=mybir.AluOpType.subtract, op1=mybir.AluOpType.mult)
                    nc.scalar.activation(
                        out=x_out[:], in_=x_out[:],
                        func=mybir.ActivationFunctionType.Relu,
                        bias=beta, scale=gamma)

                batchnorm_relu(h1, h1, sb["g1"][:], sb["bb1"][:], D)

                h2_ps = pp.tile([FEAT, B], F32, tag="h2")
                nc.tensor.matmul(out=h2_ps[:], lhsT=w2t_sb[:], rhs=h1[:],
                                 start=True, stop=True)
                h2 = singles.tile([FEAT, B], F32, tag="h2sb")
                batchnorm_relu(h2_ps, h2, sb["g2"][:], sb["bb2"][:], FEAT)

                h3_ps = pp.tile([1, B], F32, tag="h3")
                nc.tensor.matmul(out=h3_ps[:], lhsT=w3t_sb[:], rhs=h2[:MAIN, :],
                                 start=True, stop=True)
                h3 = singles.tile([1, B], F32, tag="h3sb")
                nc.vector.tensor_scalar_add(
                    out=h3[:], in0=h3_ps[:], scalar1=sb["b3"][:])
                nc.sync.dma_start(out=out[:], in_=h3[:])

    nc.compile()
    return nc


def kernel(**inputs):
    in_maps, meta = prepare(**inputs)
    nc = build_program(meta)
    res = run_bass_kernel_spmd(nc, in_maps, list(range(CORES)))
    logits = res.results[0]["out"]
    return np.ascontiguousarray(logits.T)
